# revision 27
# baseline (speedup 1.0000x reference)
"""Trainium2 Bass kernel for NeuralGraphHidden (GNN message passing).

Full-input contract: kernel(**inputs) takes the complete unsharded arrays,
shards batch dim 0 across 8 NeuronCores (data parallel), runs one SPMD Bass
program, and reassembles the full output.

Key observation: the reference masks the per-degree dense output with
(deg == arange(5)), and deg == 5 (all five edge slots used) for ~96% of
atoms, so ~96% of output rows are exactly zero.  Only atoms with deg <= 4
("active" atoms) contribute, their degrees are all in {2, 3, 4}, and each
molecule's active slots reference at most ~52 distinct atoms.

The host computes compaction *index* metadata only (active-atom lists,
referenced-atom lists, one-hot gather matrices, 0/1 degree masks -- all
integer bookkeeping); every FLOP of the tensor math runs on device:

  per core (32 molecules, 512 = 32x16 compacted slots in 4 chunks of 128):
    neighsumT = atomsref_m^T @ G_m     (TensorE; G = host one-hot of
                                        self+neighbor refs, K=64)
    sumbondT  = DVE d-reduce of pre-transposed compacted bonds
    featT     = [neighsumT; sumbondT; 1]  (321 x 512, bf16)
    Z_d       = featT^T @ Waug[d], d in {2,3,4}  (TensorE, 3 K-chunks)
    out       = sum_d relu(mask_d * Z_d)  (ScalarE relu with per-partition
                                           mask scale, read from PSUM,
                                           summed on DVE; masks disjoint)

Emission is software-pipelined (gather g | dense g-1) so TensorE does not
stall on the ScalarE PSUM->SBUF hop.  All DMAs issue from the sync/gpsimd
queues (scalar-issued DMAs take the slow software-DGE path).

Molecules that do not fit the static layout (more than WSLOT active atoms,
more than RREF referenced atoms, or an active degree outside {2,3,4}) fall
back to exact host evaluation -- never hit on this input distribution.

Padding slots have all-zero gather columns and masks; their rows are
dropped on the host anyway (scatter writes only real slots into zeros).
"""

import sys

sys.path.insert(0, "/opt/trn_rl_repo")

import numpy as np

B, A, D = 256, 128, 5
FA, FB, C = 256, 64, 256
F = FA + FB        # 320
FAUG = F + 1       # 321 (bias row)
NCORES = 8
BL = B // NCORES   # 32 molecules per core
WSLOT = 16         # compacted slots per molecule (max observed active = 12)
RREF = 64          # referenced atoms per molecule (max observed = 52)
NS = BL * WSLOT    # 512 slots per core
NCH = NS // 128    # 4 slot chunks
MPC = BL // NCH    # 8 molecules per chunk
DEGS = (2, 3, 4)   # degrees that occur among active atoms
ND = len(DEGS)

_CACHE = {}


def _build_program():
    from contextlib import ExitStack

    import concourse.bass as bass
    import concourse.tile as tile
    from concourse import bacc, mybir

    f32 = mybir.dt.float32
    bf16 = mybir.dt.bfloat16
    AF = mybir.ActivationFunctionType
    OP = mybir.AluOpType

    nc = bacc.Bacc("TRN2", target_bir_lowering=False, debug=False,
                   num_devices=NCORES)

    # atoms: per chunk 2 blocks of 128 packed referenced-atom rows
    atoms_d = nc.dram_tensor("atoms", [NCH * 2 * 128, FA], bf16,
                             kind="ExternalInput")
    g_d = nc.dram_tensor("gmat", [A, NCH * 2 * 128], bf16,
                         kind="ExternalInput")
    bondst_d = nc.dram_tensor("bondst", [FB, NS * D], bf16,
                              kind="ExternalInput")
    # W sliced to degrees 2..4: [w0 | w1] k-chunks and the 65-row tail chunk
    w01_d = nc.dram_tensor("w01", [128, 2 * ND * C], bf16,
                           kind="ExternalInput")
    w2_d = nc.dram_tensor("w2", [FB + 1, ND * C], bf16, kind="ExternalInput")
    mask_d = nc.dram_tensor("mask", [A, NCH * ND], f32, kind="ExternalInput")
    out_d = nc.dram_tensor("out", [A, NCH * C], bf16, kind="ExternalOutput")

    with tile.TileContext(nc) as tc, ExitStack() as ctx:
        consts = ctx.enter_context(tc.tile_pool(name="consts", bufs=1))
        pin = ctx.enter_context(tc.tile_pool(name="pin", bufs=NCH))
        pfeat = ctx.enter_context(tc.tile_pool(name="pfeat", bufs=NCH))
        pt = ctx.enter_context(tc.tile_pool(name="pt", bufs=2))
        pout = ctx.enter_context(tc.tile_pool(name="pout", bufs=2))
        ps_ga = ctx.enter_context(
            tc.tile_pool(name="ps_ga", bufs=NCH, space="PSUM"))
        ps_z = ctx.enter_context(
            tc.tile_pool(name="ps_z", bufs=1, space="PSUM"))

        # ---- one-time setup: loads ordered by need-time across queues ------
        # sync HW queue: atoms0 (gather0), w01 (dense0), bondst (featTbot)
        # scalar HW queue: gmat (gather0), w2, odd atoms chunks
        # gpsimd SW queue: mask, even atoms chunks (needed latest)
        atoms_t = [None] * NCH
        pin_pool = pin

        def _atoms_dma(g, eng):
            atoms_t[g] = pin_pool.tile([A, 2 * FA], bf16, name=f"atoms{g}")
            for bb in range(2):
                eng.dma_start(
                    out=atoms_t[g][:, bb * FA:(bb + 1) * FA],
                    in_=atoms_d.ap()[(g * 2 + bb) * 128:
                                     (g * 2 + bb + 1) * 128, :])

        bondst = consts.tile([FB, NS * D], bf16)
        gmat = consts.tile([A, NCH * 2 * 128], bf16)

        def _gmat_dma(g, eng):
            eng.dma_start(out=gmat[:, g * 256:(g + 1) * 256],
                          in_=g_d.ap()[:, g * 256:(g + 1) * 256])

        def _bondst_dma(g, eng):
            eng.dma_start(out=bondst[:, g * 128 * D:(g + 1) * 128 * D],
                          in_=bondst_d.ap()[:, g * 128 * D:(g + 1) * 128 * D])

        _gmat_dma(0, nc.scalar)
        _atoms_dma(0, nc.sync)
        _bondst_dma(0, nc.sync)
        _atoms_dma(1, nc.scalar)
        w01 = consts.tile([128, 2 * ND * C], bf16)
        nc.sync.dma_start(out=w01[:], in_=w01_d.ap()[:])
        w0 = w01[:, 0:ND * C]
        w1 = w01[:, ND * C:2 * ND * C]
        _gmat_dma(1, nc.scalar)
        _bondst_dma(1, nc.scalar)
        w2 = consts.tile([FB + 1, ND * C], bf16)
        nc.sync.dma_start(out=w2[:], in_=w2_d.ap()[:])
        _atoms_dma(2, nc.scalar)
        _gmat_dma(2, nc.scalar)
        _bondst_dma(2, nc.scalar)
        _atoms_dma(3, nc.scalar)
        _gmat_dma(3, nc.scalar)
        _bondst_dma(3, nc.scalar)
        mask = consts.tile([A, NCH * ND], f32)
        nc.gpsimd.dma_start(out=mask[:], in_=mask_d.ap()[:])

        # featT rows 256..320: 64 bond-sum rows + the ones bias row
        featTbot = consts.tile([FB + 1, NS], bf16)
        nc.vector.memset(featTbot[FB:FB + 1, :], 1.0)

        featT_t = [None] * NCH

        PPC = MPC // 2          # molecule pairs per chunk

        def emit_bonds(g):
            with nc.allow_low_precision(reason="bf16 bond sums"):
                nc.vector.tensor_reduce(
                    featTbot[0:FB, g * 128:(g + 1) * 128],
                    bondst[:, g * 128 * D:(g + 1) * 128 * D].rearrange(
                        "p (j d) -> p j d", d=D),
                    axis=mybir.AxisListType.X, op=OP.add)

        def emit_gather(g):
            # neighbor+self sums for this chunk's 128 slots (2 FA halves
            # side by side in one PSUM tile); contraction over the 64
            # referenced-atom rows of each molecule
            atoms4 = atoms_t[g]
            pga = ps_ga.tile([A, 256], f32)
            for h in range(2):          # close each accumulation group
                for b in range(2):      # before opening the next region
                    lhs = atoms4[:, b * FA + h * 128:b * FA + (h + 1) * 128]
                    rhs = gmat[:, (g * 2 + b) * 128:(g * 2 + b + 1) * 128]
                    nc.tensor.matmul(pga[:, h * 128:(h + 1) * 128], lhs, rhs,
                                     start=(b == 0), stop=(b == 1))
            featT_t[g] = pfeat.tile([A, 256], bf16, name=f"featT{g}")
            nc.scalar.copy(featT_t[g][:], pga[:])

        def emit_dense(g):
            # Z_d = feat @ Waug[d] for d in DEGS, then the degree select as
            # relu(mask_d * Z_d) (ScalarE, PSUM input, per-partition scale)
            # summed over the disjoint masks on DVE
            featT0 = featT_t[g][:, 0:128]
            featT1 = featT_t[g][:, 128:256]
            fb_lhs = featTbot[:, g * 128:(g + 1) * 128]
            pzA = ps_z.tile([A, 512], f32, tag="pzA", bufs=2)
            pzB = ps_z.tile([A, 256], f32, tag="pzB", bufs=2)
            for k, lhs, w in ((0, featT0, w0), (1, featT1, w1),
                              (2, fb_lhs, w2)):
                nc.tensor.matmul(pzA[:], lhs, w[:, 0:512],
                                 start=(k == 0), stop=(k == 2))
            t2 = pt.tile([A, C], bf16, name=f"t2_{g}")
            nc.scalar.activation(t2[:], pzA[:, 0:256], AF.Relu,
                                 scale=mask[:, g * ND:g * ND + 1])
            t3 = pt.tile([A, C], bf16, name=f"t3_{g}")
            with nc.allow_low_precision(reason="bf16 masked relu"):
                nc.vector.tensor_scalar(t3[:], pzA[:, 256:512],
                                        mask[:, g * ND + 1:g * ND + 2], 0.0,
                                        OP.mult, OP.max)
            for k, lhs, w in ((0, featT0, w0), (1, featT1, w1),
                              (2, fb_lhs, w2)):
                nc.tensor.matmul(pzB[:], lhs, w[:, 512:768],
                                 start=(k == 0), stop=(k == 2))
            t4 = pt.tile([A, C], bf16, name=f"t4_{g}")
            nc.scalar.activation(t4[:], pzB[:], AF.Relu,
                                 scale=mask[:, g * ND + 2:g * ND + 3])
            t23 = pt.tile([A, C], bf16, name=f"t23_{g}")
            out4 = pout.tile([A, C], bf16, name=f"out{g}")
            with nc.allow_low_precision(reason="bf16 relu sums, disjoint"):
                nc.vector.tensor_add(t23[:], t2[:], t3[:])
                nc.vector.tensor_add(out4[:], t23[:], t4[:])
            nc.sync.dma_start(out=out_d.ap()[:, g * C:(g + 1) * C],
                              in_=out4[:])

        # ---- software-pipelined emission: gather g | dense g-1 -------------
        for g in range(NCH + 1):
            if g < NCH:
                emit_bonds(g)
                emit_gather(g)
            if g >= 1:
                emit_dense(g - 1)

    nc.compile()
    return nc


def _get_nc():
    if "nc" not in _CACHE:
        _CACHE["nc"] = _build_program()
    return _CACHE["nc"]


def _prep(atoms, bonds, edges, W, b):
    """Host-side compaction index metadata + device input layouts."""
    import ml_dtypes

    atoms = np.ascontiguousarray(np.asarray(atoms, dtype=np.float32))
    bonds = np.ascontiguousarray(np.asarray(bonds, dtype=np.float32))
    edges = np.asarray(edges)
    W = np.asarray(W, dtype=np.float32)
    b = np.asarray(b, dtype=np.float32)

    deg = (edges != -1).sum(-1)                      # (B, A)
    act = deg <= D - 1                               # only these rows nonzero

    bf = ml_dtypes.bfloat16
    sel = np.zeros((B, WSLOT), dtype=np.int64)
    valid = np.zeros((B, WSLOT), dtype=bool)
    overflow = np.zeros(B, dtype=bool)
    mol_act = [None] * B
    mol_refs = [None] * B

    for m in range(B):
        idxs = np.nonzero(act[m])[0]
        if len(idxs) > WSLOT or not np.isin(deg[m][idxs], DEGS).all():
            overflow[m] = True
            mol_act[m] = idxs[:0]
            mol_refs[m] = []
            continue
        refs = {}
        for t, a in enumerate(idxs):
            for e in [a] + [e for e in edges[m, a] if e >= 0]:
                refs.setdefault(int(e), len(refs))
            sel[m, t] = a
            valid[m, t] = True
        mol_act[m] = idxs
        mol_refs[m] = sorted(refs, key=refs.get)

    # bin-pack molecules: balance referenced-atom counts across the NCH
    # chunks (8 window positions each), then split each chunk's molecules
    # into 2 gather blocks of <= 128 packed rows
    perm = np.zeros((NCORES, BL), dtype=np.int64)      # window -> local mol
    blkof = np.zeros((NCORES, BL, 2), dtype=np.int64)  # (block, row offset)
    for c in range(NCORES):
        rc = np.array([len(mol_refs[c * BL + m]) for m in range(BL)])
        order = np.argsort(-rc, kind="stable")
        chunks = [[] for _ in range(NCH)]
        sums = [0] * NCH
        for m in order:
            for i in sorted(range(NCH), key=lambda i: sums[i]):
                if len(chunks[i]) < MPC:
                    chunks[i].append(int(m))
                    sums[i] += int(rc[m])
                    break
        for g in range(NCH):
            fill = [0, 0]
            for wic, m in enumerate(chunks[g]):
                r = int(rc[m])
                bb = int(fill[1] < fill[0])
                if fill[bb] + r > 128:
                    bb = 1 - bb
                if fill[bb] + r > 128:         # cannot happen per packing
                    overflow[c * BL + m] = True
                    valid[c * BL + m] = False
                    r = 0
                perm[c, g * MPC + wic] = m
                blkof[c, g * MPC + wic] = (bb, fill[bb])
                fill[bb] += r

    atomsP = np.zeros((NCORES, NCH * 2 * 128, FA), dtype=np.float32)
    gmatP = np.zeros((NCORES, A, NCH * 2 * 128), dtype=np.float32)
    bonds_w = np.zeros((NCORES, NS, D, FB), dtype=np.float32)
    deg_w = np.full((NCORES, NS), -1, dtype=np.int64)
    for c in range(NCORES):
        for w in range(BL):
            m = int(perm[c, w])
            M = c * BL + m
            if overflow[M]:
                continue
            g, wic = w // MPC, w % MPC
            bb, off = int(blkof[c, w, 0]), int(blkof[c, w, 1])
            refs = mol_refs[M]
            base = (g * 2 + bb) * 128 + off
            atomsP[c, base:base + len(refs)] = atoms[M, refs]
            rmap = {a: off + r for r, a in enumerate(refs)}
            col0 = (g * 2 + bb) * 128
            for t, a in enumerate(mol_act[M]):
                s = wic * WSLOT + t
                for e in [int(a)] + [int(e) for e in edges[M, a] if e >= 0]:
                    gmatP[c, rmap[e], col0 + wic * WSLOT + t] += 1.0
                bonds_w[c, g * 128 + s] = bonds[M, a]
                deg_w[c, g * 128 + s] = deg[M, a]

    atoms8 = np.ascontiguousarray(atomsP).astype(bf)
    gmat8 = np.ascontiguousarray(gmatP).astype(bf)
    bondst8 = np.ascontiguousarray(
        bonds_w.transpose(0, 3, 1, 2)).reshape(NCORES, FB, NS * D).astype(bf)

    # per-chunk, per-degree 0/1 select masks (slot on partition)
    dg = deg_w.reshape(NCORES, NCH, A)
    mask8 = np.zeros((NCORES, A, NCH, ND), dtype=np.float32)
    for i, dd in enumerate(DEGS):
        mask8[:, :, :, i] = (dg == dd).transpose(0, 2, 1)
    mask8 = np.ascontiguousarray(mask8.reshape(NCORES, A, NCH * ND))

    # W sliced to the degrees that occur, bias folded as the last feat row
    waug = np.concatenate([W, b[:, None, :]], axis=1)     # (5, 321, 256)
    wdeg = waug[list(DEGS)]                               # (3, 321, 256)
    w0 = wdeg[:, 0:128, :].transpose(1, 0, 2).reshape(128, ND * C)
    w1 = wdeg[:, 128:256, :].transpose(1, 0, 2).reshape(128, ND * C)
    w2 = wdeg[:, 256:FAUG, :].transpose(1, 0, 2).reshape(FAUG - 256, ND * C)
    w01 = np.ascontiguousarray(
        np.concatenate([w0, w1], axis=1)).astype(bf)

    in_maps = [
        {
            "atoms": atoms8[c],
            "gmat": gmat8[c],
            "bondst": bondst8[c],
            "w01": w01,
            "w2": np.ascontiguousarray(w2).astype(bf),
            "mask": mask8[c],
        }
        for c in range(NCORES)
    ]
    return in_maps, sel, valid, overflow, perm


def _host_reference_rows(atoms_m, bonds_m, edges_m, W, b):
    """Exact per-molecule fallback (for molecules the layout can't hold)."""
    deg = (edges_m != -1).sum(-1)
    masked = np.concatenate([np.zeros((1, FA), np.float32), atoms_m], axis=0)
    neigh = masked[edges_m + 1]                       # (A, D, FA)
    feat = np.concatenate([atoms_m + neigh.sum(1), bonds_m.sum(1)], axis=-1)
    out = np.zeros((A, C), np.float32)
    for d in range(D):
        rows = deg == d
        if rows.any():
            out[rows] = np.maximum(feat[rows] @ W[d] + b[d], 0.0)
    return out


def run_sharded(atoms, bonds, edges, W, b, trace=False):
    """Run on the 8 NeuronCores; returns (output, BassKernelResults)."""
    from concourse.bass_utils import run_bass_kernel_spmd

    nc = _get_nc()
    in_maps, sel, valid, overflow, perm = _prep(atoms, bonds, edges, W, b)
    res = run_bass_kernel_spmd(nc, in_maps, list(range(NCORES)), trace=trace)

    out = np.zeros((B, A, C), dtype=np.float32)
    dev = np.stack([np.asarray(res.results[c]["out"], dtype=np.float32)
                    for c in range(NCORES)])
    # device layout (A=slot%128, NCH chunks, C) -> window order, then map
    # window positions back to their original molecules via perm
    dev = dev.reshape(NCORES, A, NCH, C).transpose(0, 2, 1, 3).reshape(
        NCORES, BL, WSLOT, C)
    invdev = np.zeros_like(dev)
    core_idx = np.repeat(np.arange(NCORES), BL)
    invdev[core_idx, perm.ravel()] = dev[core_idx,
                                         np.tile(np.arange(BL), NCORES)]
    dev = invdev.reshape(B, WSLOT, C)
    mm, tt = np.nonzero(valid)
    out[mm, sel[mm, tt]] = dev[mm, tt]

    if overflow.any():  # exact host fallback; never hit on this distribution
        atoms = np.asarray(atoms, dtype=np.float32)
        bonds = np.asarray(bonds, dtype=np.float32)
        edges = np.asarray(edges)
        for m in np.nonzero(overflow)[0]:
            out[m] = _host_reference_rows(atoms[m], bonds[m], edges[m],
                                          np.asarray(W, dtype=np.float32),
                                          np.asarray(b, dtype=np.float32))
    return out, res


def kernel(atoms, bonds, edges, W, b):
    out, _ = run_sharded(atoms, bonds, edges, W, b)
    return out
